# revision 16
# baseline (speedup 1.0000x reference)
"""Trainium2 Bass kernel for nn_MultiHeadEncDecAttention.

Problem (full shapes):
  x:[4,512,8,256] z:[256,512,32] w_q_w:[256,256] fc_w:[256,256] (+biases, LN params)
  q = x@w_q_w.T (+b) -> [h,v,b,s,dq]; attn = softmax(q@z^T/sqrt(dq)); out = attn@z
  o2 = concat_h(out)@fc_w.T (+b); y = LN(o2 + x)*gamma + beta

Sharding: split on n_verts (nv=8) across the 8 cores - every stage
(q-proj, attention, fc, LN) is independent per vert, so zero cross-core comms.

Per-core dataflow (v fixed, r = b*512+s in [0,2048), e = 32h+dq in [0,256)):
  qT[e,r]   = wqT.T @ xT           (PE, fp32r, PSUM->SBUF copy w/ bias)
  per pair i=(b,h):
    lgT[t,s] = zT_hb.T' @ qT_hb    (PE fp32r; [t,s] layout: 4 MMs into one 4-bank PSUM tile)
    eT       = exp(lgT/sqrt(dq))   (ACT, one [128,2048] op, bf16 out)
    av[0:33] = zA_hb.T' @ eT       (PE bf16; zA has a ones-column -> row 32 = softmax sums)
    stg      = cast av -> bf16; DMA rows 0:32 -> outcT block, row 32 -> sums[h]
  per batch b: recip = 1/sums; bcast rows; outcT *= bcast (softmax normalize)
  fc:  o2 = outcT.T' @ fcT (PE bf16) ; y = o2 + (x+fc_b) ; bn_stats/aggr
  tail: rstd = 1/sqrt(var+eps) once; y_out = (y-mu)*rstd [*gamma+beta]
"""

import sys

sys.path.insert(0, "/opt/trn_rl_repo")

from contextlib import ExitStack

import ml_dtypes
import numpy as np

import concourse.bass as bass
import concourse.tile as tile
from concourse import mybir

F32 = mybir.dt.float32
F32R = mybir.dt.float32r
BF16 = mybir.dt.bfloat16
AX = mybir.AluOpType
AF = mybir.ActivationFunctionType

N_HEAD = 8
D_Q = 32
D_IN = 256
BS = 4
SEG = 512
NV = 8
LN_EPS = 1e-5
R = BS * SEG  # 2048 rows per core
NCORES = 8
INV_TEMP = 1.0 / np.sqrt(np.float32(D_Q))

_prog_cache = {}


def _build(use_wqb: bool, use_gb: bool):
    from concourse import bacc

    nc = bacc.Bacc("TRN2", target_bir_lowering=False, debug=False)

    d_xT = nc.dram_tensor("xT", [D_IN, R], F32R, kind="ExternalInput").ap()
    d_xres = nc.dram_tensor("xres", [128, 16 * 256], F32, kind="ExternalInput").ap()
    d_zT = nc.dram_tensor("zT", [3, 128, 2048], F32R, kind="ExternalInput").ap()
    d_zA = nc.dram_tensor("zA", [4, 128, 32 * 33], BF16, kind="ExternalInput").ap()
    d_wqT = nc.dram_tensor("wqT", [D_IN, 256], F32R, kind="ExternalInput").ap()
    d_wqb = nc.dram_tensor("wqb", [128, 3], F32, kind="ExternalInput").ap()
    d_fcT = nc.dram_tensor("fcT", [256, D_IN], BF16, kind="ExternalInput").ap()
    d_gbb = nc.dram_tensor("gbb", [128, 512], F32, kind="ExternalInput").ap()
    d_y = nc.dram_tensor("y", [R, D_IN], F32, kind="ExternalOutput").ap()

    with tile.TileContext(nc) as tc, ExitStack() as ctx:
        P = ctx.enter_context  # noqa

        big = P(tc.tile_pool(name="big", bufs=1))
        psum = P(tc.tile_pool(name="psum", bufs=1, space="PSUM"))
        expp = P(tc.tile_pool(name="expp", bufs=2))
        stgp = P(tc.tile_pool(name="stgp", bufs=3))
        bcp = P(tc.tile_pool(name="bcp", bufs=2))
        smp = P(tc.tile_pool(name="smp", bufs=2))
        stp = P(tc.tile_pool(name="stp", bufs=2))
        outp = P(tc.tile_pool(name="outp", bufs=3))

        # ---- persistent SBUF tiles + input DMAs
        xT_t = [big.tile([128, R], F32R, name=f"xT{k}") for k in range(2)]
        for k in range(2):
            nc.sync.dma_start(xT_t[k][:], d_xT[128 * k : 128 * (k + 1), :])
        zT_t = [big.tile([128, 2048], F32R, name=f"zT{u}") for u in range(3)]
        for u in range(3):
            nc.sync.dma_start(zT_t[u][:], d_zT[u])
        zA_t = [big.tile([128, 32 * 33], BF16, name=f"zA{c}") for c in range(4)]
        for c in range(4):
            nc.sync.dma_start(zA_t[c][:], d_zA[c])
        wqT_t = [big.tile([128, 256], F32R, name=f"wqT{k}") for k in range(2)]
        for k in range(2):
            nc.sync.dma_start(wqT_t[k][:], d_wqT[128 * k : 128 * (k + 1), :])
        wqb_t = big.tile([128, 3], F32)
        nc.sync.dma_start(wqb_t[:], d_wqb)
        fcT_t = [big.tile([128, 256], BF16, name=f"fcT{e}") for e in range(2)]
        for e in range(2):
            nc.sync.dma_start(fcT_t[e][:], d_fcT[128 * e : 128 * (e + 1), :])
        gbb_t = big.tile([128, 512], F32)
        if use_gb:
            nc.sync.dma_start(gbb_t[:], d_gbb)
        xres_t = big.tile([128, 16 * 256], F32)
        nc.sync.dma_start(xres_t[:], d_xres)

        qT_t = [big.tile([128, R], F32R, name=f"qT{u}") for u in range(3)]
        outcT = [big.tile([128, R], BF16, name=f"outcT{e}") for e in range(2)]
        yhold = big.tile([128, 16 * 256], F32)
        mvall = big.tile([128, 32], F32)

        PAB = [psum.tile([128, 2048], F32, name=f"P{j}") for j in range(2)]

        def mm(out, lhsT, rhs, **kw):
            nc.tensor.matmul(out, lhsT, rhs, skip_group_check=True, **kw)

        # ---- q projection: qT[e,r] = wqT.T @ xT  (fp32r)
        # heads packed 3-per-tile at partition bases {0,32,64} (matmul base limit)
        for u in range(3):
            M = 96 if u < 2 else 64
            e0 = 96 * u
            T = PAB[u % 2]
            for n in range(4):
                for k in range(2):
                    mm(
                        T[0:M, 512 * n : 512 * (n + 1)],
                        wqT_t[k][:, e0 : e0 + M],
                        xT_t[k][:, 512 * n : 512 * (n + 1)],
                        start=(k == 0),
                        stop=(k == 1),
                    )
            if use_wqb:
                nc.vector.tensor_scalar(
                    qT_t[u][0:M, :], T[0:M, :], wqb_t[0:M, u : u + 1], 0.0, AX.add, AX.add
                )
            else:
                nc.vector.tensor_copy(qT_t[u][0:M, :], T[0:M, :])

        # ---- attention pairs, batch-major
        for b in range(BS):
            sums_b = smp.tile([8, 512], BF16)
            for h in range(N_HEAD):
                i = b * 8 + h
                T = PAB[i % 2]
                u, o3 = h // 3, 32 * (h % 3)
                # logits^T [t,s]: 4 MMs (tchunks) fp32r
                for c in range(4):
                    mm(
                        T[:, 512 * c : 512 * (c + 1)],
                        zT_t[u][o3 : o3 + 32, 512 * b + 128 * c : 512 * b + 128 * (c + 1)],
                        qT_t[u][o3 : o3 + 32, 512 * b : 512 * (b + 1)],
                        start=True,
                        stop=True,
                    )
                expt = expp.tile([128, 2048], BF16)
                nc.scalar.activation(expt[:], T[:], AF.Exp, scale=float(INV_TEMP))
                # AV: [33,512] = zA.T' @ expT ; row 32 = column sums (ones col)
                for c in range(4):
                    mm(
                        T[0:33, 0:512],
                        zA_t[c][:, 33 * i : 33 * (i + 1)],
                        expt[:, 512 * c : 512 * (c + 1)],
                        start=(c == 0),
                        stop=(c == 3),
                    )
                stg = stgp.tile([33, 512], BF16)
                nc.vector.tensor_copy(stg[:], T[0:33, 0:512])
                po = 32 * (h % 4)
                nc.sync.dma_start(
                    outcT[h // 4][po : po + 32, 512 * b : 512 * (b + 1)], stg[0:32, :]
                )
                nc.sync.dma_start(sums_b[h : h + 1, :], stg[32:33, :])

            # softmax denominators for this batch
            recf = smp.tile([8, 512], F32)
            nc.vector.reciprocal(recf[:], sums_b[:])
            recb = smp.tile([8, 512], BF16)
            nc.vector.tensor_copy(recb[:], recf[:])
            for h in range(N_HEAD):
                po = 32 * (h % 4)
                scr = bcp.tile([1, 512], BF16, tag="scr")
                nc.sync.dma_start(scr[:], recb[h : h + 1, :])
                bc = bcp.tile([128, 512], BF16)
                nc.gpsimd.partition_broadcast(bc[:], scr[:], channels=128)
                sl = outcT[h // 4][po : po + 32, 512 * b : 512 * (b + 1)]
                nc.vector.tensor_tensor(sl, sl, bc[po : po + 32, :], AX.mult)

            # fc + residual + LN stats for this batch
            for sc in range(4):
                ci = 4 * b + sc
                T2 = PAB[sc % 2]
                reg = T2[:, 1024:1280]
                for e in range(2):
                    mm(
                        reg,
                        outcT[e][:, 512 * b + 128 * sc : 512 * b + 128 * (sc + 1)],
                        fcT_t[e][:],
                        start=(e == 0),
                        stop=(e == 1),
                    )
                ysl = yhold[:, 256 * ci : 256 * (ci + 1)]
                nc.vector.tensor_tensor(
                    ysl, reg, xres_t[:, 256 * ci : 256 * (ci + 1)], AX.add
                )
                st6 = stp.tile([128, 6], F32)
                nc.vector.bn_stats(st6[:], ysl)
                nc.vector.bn_aggr(mvall[:, 2 * ci : 2 * ci + 2], st6[:])

        # ---- tail: one sqrt for all 16 chunks, then normalize + store
        eps_t = big.tile([128, 1], F32)
        nc.vector.memset(eps_t[:], float(LN_EPS))
        mv3 = mvall.rearrange("p (c two) -> p c two", two=2)
        rstd = big.tile([128, 16], F32)
        nc.scalar.activation(rstd[:], mv3[:, :, 1:2], AF.Sqrt, bias=eps_t[:])
        rstr = big.tile([128, 16], F32)
        nc.vector.reciprocal(rstr[:], rstd[:])
        for ci in range(16):
            ysl = yhold[:, 256 * ci : 256 * (ci + 1)]
            yo = outp.tile([128, 256], F32)
            if use_gb:
                t1 = outp.tile([128, 256], F32, tag="t1")
                nc.vector.scalar_tensor_tensor(
                    t1[:], ysl, mvall[:, 2 * ci : 2 * ci + 1], gbb_t[:, 0:256],
                    AX.subtract, AX.mult,
                )
                nc.vector.scalar_tensor_tensor(
                    yo[:], t1[:], rstr[:, ci : ci + 1], gbb_t[:, 256:512],
                    AX.mult, AX.add,
                )
            else:
                nc.vector.tensor_scalar(
                    yo[:], ysl, mvall[:, 2 * ci : 2 * ci + 1], rstr[:, ci : ci + 1],
                    AX.subtract, AX.mult,
                )
            nc.sync.dma_start(d_y[128 * ci : 128 * (ci + 1), :], yo[:])

    nc.compile()
    return nc


def _prep_core(x, z, fc_b, v):
    """Build the per-core input map (host-side layout packing) for vert v."""
    bf = ml_dtypes.bfloat16
    xv = np.ascontiguousarray(x[:, :, v, :]).reshape(R, D_IN)  # [r, d]
    xT = np.ascontiguousarray(xv.T)  # [d, r] f32
    xres = np.ascontiguousarray(
        (xv + fc_b[None, :]).reshape(16, 128, 256).transpose(1, 0, 2).reshape(128, 16 * 256)
    )
    zv = z.reshape(N_HEAD, NV, BS, SEG, D_Q)[:, v]  # [h, b, t, d]
    zTp = zv.transpose(0, 1, 3, 2)  # [h, b, d, t]
    zT = np.zeros((3, 128, 4, 512), np.float32)
    for h in range(N_HEAD):
        for b in range(BS):
            zT[h // 3, 32 * (h % 3) : 32 * (h % 3) + 32, b] = zTp[h, b]
    zT = np.ascontiguousarray(zT.reshape(3, 128, 2048))
    zA = np.zeros((4, 128, 32 * 33), bf)
    za_full = np.concatenate(
        [zv, np.ones((N_HEAD, BS, SEG, 1), np.float32)], axis=-1
    ).astype(bf)  # [h, b, t, 33]
    for b in range(BS):
        for h in range(N_HEAD):
            i = b * 8 + h
            for c in range(4):
                zA[c, :, 33 * i : 33 * (i + 1)] = za_full[h, b, 128 * c : 128 * (c + 1), :]
    return {"xT": xT, "xres": xres, "zT": zT, "zA": zA}


def kernel(x, z, w_q_w, w_q_b, fc_w, fc_b, ln_gamma, ln_beta, _trace=False, _tmpdir=None):
    from concourse.bass_utils import run_bass_kernel_spmd

    x = np.asarray(x, np.float32)
    z = np.asarray(z, np.float32)
    w_q_w = np.asarray(w_q_w, np.float32)
    w_q_b = np.asarray(w_q_b, np.float32)
    fc_w = np.asarray(fc_w, np.float32)
    fc_b = np.asarray(fc_b, np.float32)
    ln_gamma = np.asarray(ln_gamma, np.float32)
    ln_beta = np.asarray(ln_beta, np.float32)

    use_wqb = bool(np.any(w_q_b != 0.0))
    use_gb = bool(np.any(ln_gamma != 1.0) or np.any(ln_beta != 0.0))

    key = (use_wqb, use_gb)
    if key not in _prog_cache:
        _prog_cache[key] = _build(use_wqb, use_gb)
    nc = _prog_cache[key]

    bf = ml_dtypes.bfloat16
    wqb_p = np.zeros((128, 3), np.float32)
    for u in range(3):
        n = 96 if u < 2 else 64
        wqb_p[0:n, u] = w_q_b[96 * u : 96 * u + n]
    shared = {
        "wqT": np.ascontiguousarray(w_q_w.T),  # [d_in, e] f32
        "wqb": wqb_p,
        "fcT": np.ascontiguousarray(fc_w.T).astype(bf),  # [e, d_in]
        "gbb": np.ascontiguousarray(
            np.concatenate(
                [
                    np.broadcast_to(ln_gamma, (128, 256)),
                    np.broadcast_to(ln_beta, (128, 256)),
                ],
                axis=1,
            )
        ),
    }
    in_maps = []
    for v in range(NCORES):
        m = dict(shared)
        m.update(_prep_core(x, z, fc_b, v))
        in_maps.append(m)

    res = run_bass_kernel_spmd(
        nc,
        in_maps,
        core_ids=list(range(NCORES)),
        trace=_trace,
        tmpdir=_tmpdir,
    )
    out = np.empty((BS, SEG, NV, D_IN), np.float32)
    for v in range(NCORES):
        out[:, :, v, :] = res.results[v]["y"].reshape(BS, SEG, D_IN)
    kernel._last_result = res
    return out


# revision 21
# speedup vs baseline: 1.4454x; 1.4454x over previous
"""Trainium2 Bass kernel for nn_MultiHeadEncDecAttention.

Problem (full shapes):
  x:[4,512,8,256] z:[256,512,32] w_q_w:[256,256] fc_w:[256,256] (+biases, LN params)
  q = x@w_q_w.T (+b) -> [h,v,b,s,dq]; attn = softmax(q@z^T/sqrt(dq)); out = attn@z
  o2 = concat_h(out)@fc_w.T (+b); y = LN(o2 + x)*gamma + beta

Sharding: split on n_verts (nv=8) across the 8 cores - every stage
(q-proj, attention, fc, LN) is independent per vert, so zero cross-core comms.

Per-core dataflow (v fixed, r = b*512+s in [0,2048), e = 32h+dq in [0,256)):
  qT[e,r]   = wqT.T @ xT           (PE, fp32r, PSUM->SBUF copy w/ bias)
  per pair i=(b,h):
    lgT[t,s] = zT_hb.T' @ qT_hb    (PE fp32r; [t,s] layout: 4 MMs into one 4-bank PSUM tile)
    eT       = exp(lgT/sqrt(dq))   (ACT, one [128,2048] op, bf16 out)
    av[0:33] = zA_hb.T' @ eT       (PE bf16; zA has a ones-column -> row 32 = softmax sums)
    stg      = cast av -> bf16; DMA rows 0:32 -> outcT block, row 32 -> sums[h]
  per batch b: recip = 1/sums; bcast rows; outcT *= bcast (softmax normalize)
  fc:  o2 = outcT.T' @ fcT (PE bf16) ; y = o2 + (x+fc_b) ; bn_stats/aggr
  tail: rstd = 1/sqrt(var+eps) once; y_out = (y-mu)*rstd [*gamma+beta]
"""

import sys

sys.path.insert(0, "/opt/trn_rl_repo")

from contextlib import ExitStack

import ml_dtypes
import numpy as np

import concourse.bass as bass
import concourse.tile as tile
from concourse import mybir

F32 = mybir.dt.float32
F32R = mybir.dt.float32r
BF16 = mybir.dt.bfloat16
AX = mybir.AluOpType
AF = mybir.ActivationFunctionType

N_HEAD = 8
D_Q = 32
D_IN = 256
BS = 4
SEG = 512
NV = 8
LN_EPS = 1e-5
R = BS * SEG  # 2048 rows per core
NCORES = 8
INV_TEMP = 1.0 / np.sqrt(np.float32(D_Q))

_prog_cache = {}


def _build(use_wqb: bool, use_gb: bool):
    from concourse import bacc

    nc = bacc.Bacc("TRN2", target_bir_lowering=False, debug=False)

    d_xT = nc.dram_tensor("xT", [D_IN, R], F32R, kind="ExternalInput").ap()
    d_xres = nc.dram_tensor("xres", [128, 16 * 256], F32, kind="ExternalInput").ap()
    d_zT = nc.dram_tensor("zT", [3, 128, 2048], F32R, kind="ExternalInput").ap()
    d_zA = nc.dram_tensor("zA", [4, 128, 32 * 33], BF16, kind="ExternalInput").ap()
    d_wqT = nc.dram_tensor("wqT", [D_IN, 256], F32R, kind="ExternalInput").ap()
    d_wqb = nc.dram_tensor("wqb", [128, 3], F32, kind="ExternalInput").ap()
    d_fcT = nc.dram_tensor("fcT", [256, D_IN], BF16, kind="ExternalInput").ap()
    d_gbb = nc.dram_tensor("gbb", [128, 512], F32, kind="ExternalInput").ap()
    d_ind4 = nc.dram_tensor("ind4", [4, 128], BF16, kind="ExternalInput").ap()
    d_y = nc.dram_tensor("y", [R, D_IN], F32, kind="ExternalOutput").ap()

    with tile.TileContext(nc) as tc, ExitStack() as ctx:
        P = ctx.enter_context  # noqa

        big = P(tc.tile_pool(name="big", bufs=1))
        lgp = P(tc.tile_pool(name="lgp", bufs=3, space="PSUM"))
        avp = P(tc.tile_pool(name="avp", bufs=2, space="PSUM"))
        expp = P(tc.tile_pool(name="expp", bufs=2))
        stgp = P(tc.tile_pool(name="stgp", bufs=3))
        smp = P(tc.tile_pool(name="smp", bufs=2))
        stp = P(tc.tile_pool(name="stp", bufs=2))
        outp = P(tc.tile_pool(name="outp", bufs=3))

        # ---- persistent SBUF tiles + input DMAs
        xT_t = [big.tile([128, R], F32R, name=f"xT{k}") for k in range(2)]
        for k in range(2):
            nc.sync.dma_start(xT_t[k][:], d_xT[128 * k : 128 * (k + 1), :])
        zT_t = [big.tile([128, 2048], F32R, name=f"zT{u}") for u in range(3)]
        for u in range(3):
            nc.sync.dma_start(zT_t[u][:], d_zT[u])
        zA_t = [big.tile([128, 32 * 33], BF16, name=f"zA{c}") for c in range(4)]
        for c in range(4):
            nc.sync.dma_start(zA_t[c][:], d_zA[c])
        wqT_t = [big.tile([128, 256], F32R, name=f"wqT{k}") for k in range(2)]
        for k in range(2):
            nc.sync.dma_start(wqT_t[k][:], d_wqT[128 * k : 128 * (k + 1), :])
        wqb_t = big.tile([128, 3], F32)
        nc.sync.dma_start(wqb_t[:], d_wqb)
        fcT_t = [big.tile([128, 256], BF16, name=f"fcT{e}") for e in range(2)]
        for e in range(2):
            nc.sync.dma_start(fcT_t[e][:], d_fcT[128 * e : 128 * (e + 1), :])
        gbb_t = big.tile([128, 512], F32)
        if use_gb:
            nc.sync.dma_start(gbb_t[:], d_gbb)
        xres_t = big.tile([128, 16 * 256], F32)
        nc.sync.dma_start(xres_t[:], d_xres)
        ind4_t = big.tile([4, 128], BF16)
        nc.sync.dma_start(ind4_t[:], d_ind4)

        qT_t = [big.tile([128, R], F32R, name=f"qT{u}") for u in range(3)]
        outcT = [big.tile([128, R], BF16, name=f"outcT{e}") for e in range(2)]
        yhold = big.tile([128, 16 * 256], F32)
        mvall = big.tile([128, 32], F32)

        def mm(out, lhsT, rhs, **kw):
            nc.tensor.matmul(out, lhsT, rhs, skip_group_check=True, **kw)

        # ---- q projection: qT[e,r] = wqT.T @ xT  (fp32r)
        # heads packed 3-per-tile at partition bases {0,32,64} (matmul base limit)
        for u in range(3):
            M = 96 if u < 2 else 64
            e0 = 96 * u
            for half in range(2):
                qp = lgp.tile([128, 1024], F32, tag="lg", name="qp")
                for j in range(2):
                    n = 2 * half + j
                    for k in range(2):
                        mm(
                            qp[0:M, 512 * j : 512 * (j + 1)],
                            wqT_t[k][:, e0 : e0 + M],
                            xT_t[k][:, 512 * n : 512 * (n + 1)],
                            start=(k == 0),
                            stop=(k == 1),
                        )
                dst = qT_t[u][0:M, 1024 * half : 1024 * (half + 1)]
                if use_wqb:
                    nc.vector.tensor_scalar(
                        dst, qp[0:M, :], wqb_t[0:M, u : u + 1], 0.0, AX.add, AX.add
                    )
                else:
                    nc.scalar.copy(dst, qp[0:M, :])

        # ---- attention pairs, batch-major
        for b in range(BS):
            sums_b = smp.tile([8, 512], BF16)
            for h in range(N_HEAD):
                i = b * 8 + h
                u, o3 = h // 3, 32 * (h % 3)
                expt = expp.tile([128, 2048], BF16)
                # logits^T [t,s] in two 2-bank halves (fp32r), exp each half
                for half in range(2):
                    lt = lgp.tile([128, 1024], F32, tag="lg", name="lt")
                    for j in range(2):
                        c = 2 * half + j
                        mm(
                            lt[:, 512 * j : 512 * (j + 1)],
                            zT_t[u][o3 : o3 + 32, 512 * b + 128 * c : 512 * b + 128 * (c + 1)],
                            qT_t[u][o3 : o3 + 32, 512 * b : 512 * (b + 1)],
                            start=True,
                            stop=True,
                        )
                    nc.scalar.activation(
                        expt[:, 1024 * half : 1024 * (half + 1)],
                        lt[:],
                        AF.Exp,
                        scale=float(INV_TEMP),
                    )
                # AV: [33,512] = zA.T' @ expT ; row 32 = column sums (ones col)
                av = avp.tile([33, 512], F32, tag="avb", name="av")
                for c in range(4):
                    mm(
                        av[:],
                        zA_t[c][:, 33 * i : 33 * (i + 1)],
                        expt[:, 512 * c : 512 * (c + 1)],
                        start=(c == 0),
                        stop=(c == 3),
                    )
                stg = stgp.tile([33, 512], BF16)
                nc.vector.tensor_copy(stg[:], av[:])
                po = 32 * (h % 4)
                nc.sync.dma_start(
                    outcT[h // 4][po : po + 32, 512 * b : 512 * (b + 1)], stg[0:32, :]
                )
                nc.sync.dma_start(sums_b[h : h + 1, :], stg[32:33, :])

            # softmax denominators for this batch: 1/sums, bcast via PE rank-4
            sumf = smp.tile([8, 512], F32)
            nc.vector.tensor_copy(sumf[:], sums_b[:])
            recf = smp.tile([8, 512], F32)
            nc.vector.reciprocal_approx_fast(recf[:], sumf[:])
            recb = smp.tile([8, 512], BF16)
            nc.vector.tensor_copy(recb[:], recf[:])
            recb1 = smp.tile([4, 512], BF16)
            nc.sync.dma_start(recb1[:], recb[4:8, :])
            B_e = []
            for e in range(2):
                Bt = avp.tile([128, 512], F32, tag="avb", name="Bt")
                rhs = recb[0:4, :] if e == 0 else recb1[:]
                mm(Bt[:], ind4_t[:], rhs, start=True, stop=True)
                B_e.append(Bt)
            for h in range(N_HEAD):
                po = 32 * (h % 4)
                sl = outcT[h // 4][po : po + 32, 512 * b : 512 * (b + 1)]
                nc.vector.tensor_tensor(sl, sl, B_e[h // 4][po : po + 32, :], AX.mult)

            # fc + residual + LN stats for this batch
            for sc in range(4):
                ci = 4 * b + sc
                reg = avp.tile([128, 256], F32, tag="avb", name="fcp")
                for e in range(2):
                    mm(
                        reg[:],
                        outcT[e][:, 512 * b + 128 * sc : 512 * b + 128 * (sc + 1)],
                        fcT_t[e][:],
                        start=(e == 0),
                        stop=(e == 1),
                    )
                ysl = yhold[:, 256 * ci : 256 * (ci + 1)]
                nc.vector.tensor_tensor(
                    ysl, reg[:], xres_t[:, 256 * ci : 256 * (ci + 1)], AX.add
                )
                st6 = stp.tile([128, 6], F32)
                nc.vector.bn_stats(st6[:], ysl)
                nc.vector.bn_aggr(mvall[:, 2 * ci : 2 * ci + 2], st6[:])

        # ---- tail: one sqrt for all 16 chunks, then normalize + store
        eps_t = big.tile([128, 1], F32)
        nc.vector.memset(eps_t[:], float(LN_EPS))
        mv3 = mvall.rearrange("p (c two) -> p c two", two=2)
        rstd = big.tile([128, 16], F32)
        nc.scalar.activation(rstd[:], mv3[:, :, 1:2], AF.Sqrt, bias=eps_t[:])
        rstr = big.tile([128, 16], F32)
        nc.vector.reciprocal(rstr[:], rstd[:])
        for ci in range(16):
            ysl = yhold[:, 256 * ci : 256 * (ci + 1)]
            yo = outp.tile([128, 256], F32)
            if use_gb:
                t1 = outp.tile([128, 256], F32, tag="t1")
                nc.vector.scalar_tensor_tensor(
                    t1[:], ysl, mvall[:, 2 * ci : 2 * ci + 1], gbb_t[:, 0:256],
                    AX.subtract, AX.mult,
                )
                nc.vector.scalar_tensor_tensor(
                    yo[:], t1[:], rstr[:, ci : ci + 1], gbb_t[:, 256:512],
                    AX.mult, AX.add,
                )
            else:
                nc.vector.tensor_scalar(
                    yo[:], ysl, mvall[:, 2 * ci : 2 * ci + 1], rstr[:, ci : ci + 1],
                    AX.subtract, AX.mult,
                )
            nc.sync.dma_start(d_y[128 * ci : 128 * (ci + 1), :], yo[:])

    nc.compile()
    return nc


def _prep_core(x, z, fc_b, v):
    """Build the per-core input map (host-side layout packing) for vert v."""
    bf = ml_dtypes.bfloat16
    xv = np.ascontiguousarray(x[:, :, v, :]).reshape(R, D_IN)  # [r, d]
    xT = np.ascontiguousarray(xv.T)  # [d, r] f32
    xres = np.ascontiguousarray(
        (xv + fc_b[None, :]).reshape(16, 128, 256).transpose(1, 0, 2).reshape(128, 16 * 256)
    )
    zv = z.reshape(N_HEAD, NV, BS, SEG, D_Q)[:, v]  # [h, b, t, d]
    zTp = zv.transpose(0, 1, 3, 2)  # [h, b, d, t]
    zT = np.zeros((3, 128, 4, 512), np.float32)
    for h in range(N_HEAD):
        for b in range(BS):
            zT[h // 3, 32 * (h % 3) : 32 * (h % 3) + 32, b] = zTp[h, b]
    zT = np.ascontiguousarray(zT.reshape(3, 128, 2048))
    zA = np.zeros((4, 128, 32 * 33), bf)
    za_full = np.concatenate(
        [zv, np.ones((N_HEAD, BS, SEG, 1), np.float32)], axis=-1
    ).astype(bf)  # [h, b, t, 33]
    for b in range(BS):
        for h in range(N_HEAD):
            i = b * 8 + h
            for c in range(4):
                zA[c, :, 33 * i : 33 * (i + 1)] = za_full[h, b, 128 * c : 128 * (c + 1), :]
    return {"xT": xT, "xres": xres, "zT": zT, "zA": zA}


def kernel(x, z, w_q_w, w_q_b, fc_w, fc_b, ln_gamma, ln_beta, _trace=False, _tmpdir=None):
    from concourse.bass_utils import run_bass_kernel_spmd

    x = np.asarray(x, np.float32)
    z = np.asarray(z, np.float32)
    w_q_w = np.asarray(w_q_w, np.float32)
    w_q_b = np.asarray(w_q_b, np.float32)
    fc_w = np.asarray(fc_w, np.float32)
    fc_b = np.asarray(fc_b, np.float32)
    ln_gamma = np.asarray(ln_gamma, np.float32)
    ln_beta = np.asarray(ln_beta, np.float32)

    use_wqb = bool(np.any(w_q_b != 0.0))
    use_gb = bool(np.any(ln_gamma != 1.0) or np.any(ln_beta != 0.0))

    key = (use_wqb, use_gb)
    if key not in _prog_cache:
        _prog_cache[key] = _build(use_wqb, use_gb)
    nc = _prog_cache[key]

    bf = ml_dtypes.bfloat16
    wqb_p = np.zeros((128, 3), np.float32)
    for u in range(3):
        n = 96 if u < 2 else 64
        wqb_p[0:n, u] = w_q_b[96 * u : 96 * u + n]
    ind4 = np.zeros((4, 128), ml_dtypes.bfloat16)
    for k in range(4):
        ind4[k, 32 * k : 32 * (k + 1)] = 1.0
    shared = {
        "wqT": np.ascontiguousarray(w_q_w.T),  # [d_in, e] f32
        "wqb": wqb_p,
        "ind4": ind4,
        "fcT": np.ascontiguousarray(fc_w.T).astype(bf),  # [e, d_in]
        "gbb": np.ascontiguousarray(
            np.concatenate(
                [
                    np.broadcast_to(ln_gamma, (128, 256)),
                    np.broadcast_to(ln_beta, (128, 256)),
                ],
                axis=1,
            )
        ),
    }
    in_maps = []
    for v in range(NCORES):
        m = dict(shared)
        m.update(_prep_core(x, z, fc_b, v))
        in_maps.append(m)

    res = run_bass_kernel_spmd(
        nc,
        in_maps,
        core_ids=list(range(NCORES)),
        trace=_trace,
        tmpdir=_tmpdir,
    )
    out = np.empty((BS, SEG, NV, D_IN), np.float32)
    for v in range(NCORES):
        out[:, :, v, :] = res.results[v]["y"].reshape(BS, SEG, D_IN)
    kernel._last_result = res
    return out
